# revision 6
# baseline (speedup 1.0000x reference)
"""Continuous positional bias kernel for Trainium2 (8 NeuronCores).

Reference computation (per batch b):
    rel[q,k,:] = query_coords[b,q,:] - key_coords[b,k,:]        (2 coords)
    h1 = relu(rel @ w1 + b1)      # (Nq,Nk,128)
    h2 = relu(h1 @ w2 + b2)       # (Nq,Nk,128)
    out[b,:,q,k] = (h2 @ w3 + b3).T  # (heads=8, Nq, Nk)

Key restructuring: since layer 1 is linear in rel = q - k,
    w1^T rel + b1 = (w1^T q + b1) + (-w1^T k) = beta[:,q] + gamma[:,k]
beta (128 x Nq) and gamma (128 x Nk) are tiny and computed on host.
On device, per (q, k-tile-of-512):
    h1 = relu(gamma_tile + beta_col)      one tensor_scalar op (add, max)
    h2 = relu(w2^T h1 + b2)               PE matmul (f32r) + ACT relu from PSUM
    out = w3^T h2                         PE matmul (f32r), w3 zero-padded to
                                          [128,32] and col-tiled so 4 k-tiles
                                          pack one full [128,512] PSUM bank
    one DVE copy PSUM->SBUF staging per 4 tiles, large strided DMAs out.

Sharding: 8 cores x (batch, 256 queries). All weights replicated.
"""

import numpy as np

B, NQ, NK, H, HD = 2, 1024, 1024, 8, 128
NCORES = 8
CPB = NCORES // B          # cores per batch = 4
QPC = NQ // CPB            # queries per core = 256
KT = 512                   # k-tile (matmul moving free dim)
KH = NK // KT              # k halves = 2
NGR = 16                   # groups per staging round (group = 2 q = 4 tiles)
RQ = 2 * NGR               # queries per staging round = 32
W3P = 32                   # w3 padded to 32 output columns (col-group width)

# Engine assignment per tile j in a group of 4 (tunable):
#   h1 (SBUF->SBUF tensor_scalar): "v"=vector(DVE), "g"=gpsimd, "s"=scalar(ACT)
#   h2 (PSUM->SBUF relu+bias):     "s"=scalar(ACT), "v"=vector(DVE)
H1_ENG = ["g", "g", "g", "v"]
H2_ENG = ["s", "s", "s", "v"]

_CACHE = {}


def _build_nc():
    from contextlib import ExitStack

    import concourse.bass as bass
    import concourse.tile as tile
    from concourse import bacc, mybir
    from concourse.alu_op_type import AluOpType

    f32 = mybir.dt.float32
    f32r = mybir.dt.float32r
    bf16 = mybir.dt.bfloat16
    Relu = mybir.ActivationFunctionType.Relu

    nc = bacc.Bacc(
        "TRN2",
        target_bir_lowering=False,
        debug=False,
        enable_asserts=True,
        num_devices=NCORES,
    )

    gamma_d = nc.dram_tensor("gamma", (HD, NK), f32, kind="ExternalInput").ap()
    beta_d = nc.dram_tensor("beta", (HD, QPC), f32, kind="ExternalInput").ap()
    w2_d = nc.dram_tensor("w2", (HD, HD), f32, kind="ExternalInput").ap()
    w3p_d = nc.dram_tensor("w3p", (HD, W3P), f32, kind="ExternalInput").ap()
    b2_d = nc.dram_tensor("b2", (HD, 1), f32, kind="ExternalInput").ap()
    out_d = nc.dram_tensor("out", (H, QPC, NK), f32, kind="ExternalOutput").ap()

    with tile.TileContext(nc) as tc:
        with ExitStack() as ctx:
            consts = ctx.enter_context(tc.tile_pool(name="consts", bufs=1))
            h1p = ctx.enter_context(tc.tile_pool(name="h1p", bufs=4))
            h2p = ctx.enter_context(tc.tile_pool(name="h2p", bufs=4))
            stagep = ctx.enter_context(tc.tile_pool(name="stagep", bufs=2))
            ps2 = ctx.enter_context(tc.tile_pool(name="ps2", bufs=4, space="PSUM"))
            ps3 = ctx.enter_context(tc.tile_pool(name="ps3", bufs=2, space="PSUM"))

            gamma = consts.tile([HD, NK], f32)
            nc.sync.dma_start(gamma, gamma_d)
            beta = consts.tile([HD, QPC], f32)
            nc.sync.dma_start(beta, beta_d)
            w2 = consts.tile([HD, HD], f32)
            nc.sync.dma_start(w2, w2_d)
            w3p = consts.tile([HD, W3P], f32)
            nc.sync.dma_start(w3p, w3p_d)
            b2 = consts.tile([HD, 1], f32)
            nc.sync.dma_start(b2, b2_d)

            # walrus requires f32r matmul operands to be *produced* as f32r
            # (rounded on write), so convert weights once and write h1/h2
            # tiles as f32r directly.
            w2r = consts.tile([HD, HD], f32r)
            nc.vector.tensor_copy(w2r, w2)
            # L3 runs in bf16: f32r matmuls reject col-tiling (walrus
            # s3d3_mm_valid_dst_partition), bf16 ones accept it, and bf16
            # rounding only affects the final 8-head projection input.
            w3pr = consts.tile([HD, W3P], bf16)
            nc.vector.tensor_copy(w3pr, w3p)

            def eng(code):
                return {"v": nc.vector, "g": nc.gpsimd, "s": nc.scalar}[code]

            nrounds = QPC // RQ
            for r in range(nrounds):
                q0 = r * RQ
                stage = stagep.tile([128, NGR * KT], f32, tag="stage")
                for g in range(NGR):
                    q = q0 + 2 * g
                    p3 = ps3.tile([128, KT], f32, tag="p3")
                    for j in range(4):
                        qq = q + (j // 2)
                        kh = j % 2
                        h1 = h1p.tile([HD, KT], f32r, tag="h1")
                        eng(H1_ENG[j]).tensor_scalar(
                            h1,
                            gamma[:, kh * KT:(kh + 1) * KT],
                            beta[:, qq:qq + 1],
                            0.0,
                            AluOpType.add,
                            AluOpType.max,
                        )
                        p2 = ps2.tile([HD, KT], f32, tag="p2")
                        nc.tensor.matmul(p2, w2r, h1, start=True, stop=True)
                        h2 = h2p.tile([HD, KT], bf16, tag="h2")
                        if H2_ENG[j] == "s":
                            nc.scalar.activation(h2, p2, Relu, bias=b2)
                        else:
                            eng(H2_ENG[j]).tensor_scalar(
                                h2, p2, b2, 0.0, AluOpType.add, AluOpType.max
                            )
                        nc.tensor.matmul(
                            p3[32 * j:32 * (j + 1), :],
                            w3pr,
                            h2,
                            start=True,
                            stop=True,
                            tile_position=(0, 32 * j),
                        )
                    nc.vector.tensor_copy(stage[:, g * KT:(g + 1) * KT], p3)
                # Staging layout: free slot g holds the 4-tile group for
                # queries (q0+2g, q0+2g+1); partition block 32j..32j+8 holds
                # heads for (q offset j//2, k half j%2).  DMA per block j:
                # dest[n, g, k] = out[n, q0 + 2g + j//2, (j%2)*512 + k]
                for j in range(4):
                    dest = bass.AP(
                        tensor=out_d.tensor,
                        offset=out_d.offset
                        + (q0 + (j // 2)) * NK
                        + (j % 2) * KT,
                        ap=[[QPC * NK, H], [2 * NK, NGR], [1, KT]],
                    )
                    nc.sync.dma_start(dest, stage[32 * j:32 * j + H, :])

    nc.compile()
    return nc


def _get_nc():
    if "nc" not in _CACHE:
        _CACHE["nc"] = _build_nc()
    return _CACHE["nc"]


def make_in_maps(query_coords, key_coords, w1, b1, w2, b2, w3):
    """Host-side shard prep: per-core gamma/beta + replicated weights."""
    qc = np.asarray(query_coords, np.float32)
    kc = np.asarray(key_coords, np.float32)
    w1 = np.asarray(w1, np.float32)
    b1 = np.asarray(b1, np.float32)
    w2 = np.asarray(w2, np.float32)
    b2 = np.asarray(b2, np.float32)
    w3 = np.asarray(w3, np.float32)

    w3p = np.zeros((HD, W3P), np.float32)
    w3p[:, :H] = w3
    b2c = np.ascontiguousarray(b2.reshape(HD, 1))
    w2c = np.ascontiguousarray(w2)

    in_maps = []
    for c in range(NCORES):
        b = c // CPB
        q0 = (c % CPB) * QPC
        gamma = np.ascontiguousarray(-(kc[b] @ w1).T)            # (128, NK)
        beta = np.ascontiguousarray(
            (qc[b, q0:q0 + QPC] @ w1).T + b1[:, None]            # (128, QPC)
        )
        in_maps.append(
            {"gamma": gamma, "beta": beta, "w2": w2c, "w3p": w3p, "b2": b2c}
        )
    return in_maps


def assemble_output(results, b3):
    """Gather per-core [H, QPC, NK] results into (B, H, NQ, NK)."""
    b3 = np.asarray(b3, np.float32)
    out = np.empty((B, H, NQ, NK), np.float32)
    for c in range(NCORES):
        b = c // CPB
        q0 = (c % CPB) * QPC
        out[b, :, q0:q0 + QPC, :] = results[c]["out"]
    if np.any(b3):
        out += b3.reshape(1, H, 1, 1)
    return out


def kernel(**inputs):
    from concourse.bass_utils import run_bass_kernel_spmd

    in_maps = make_in_maps(
        inputs["query_coords"],
        inputs["key_coords"],
        inputs["w1"],
        inputs["b1"],
        inputs["w2"],
        inputs["b2"],
        inputs["w3"],
    )
    nc = _get_nc()
    res = run_bass_kernel_spmd(nc, in_maps, list(range(NCORES)))
    return assemble_output(res.results, inputs["b3"])


# revision 9
# speedup vs baseline: 9.0985x; 9.0985x over previous
"""Continuous positional bias kernel for Trainium2 (8 NeuronCores).

Reference computation (per batch b):
    rel[q,k,:] = query_coords[b,q,:] - key_coords[b,k,:]        (2 coords)
    h1 = relu(rel @ w1 + b1)      # (Nq,Nk,128)
    h2 = relu(h1 @ w2 + b2)       # (Nq,Nk,128)
    out[b,:,q,k] = (h2 @ w3 + b3).T  # (heads=8, Nq, Nk)

Key restructuring: since layer 1 is linear in rel = q - k,
    w1^T rel + b1 = (w1^T q + b1) + (-w1^T k) = beta[:,q] + gamma[:,k]
beta (128 x Nq) and gamma (128 x Nk) are tiny and computed on host.
On device, per (q, k-tile-of-512):
    h1 = relu(gamma_tile + beta_col)      one tensor_scalar op (add, max)
    h2 = relu(w2^T h1 + b2)               PE matmul (f32r) + ACT relu from PSUM
    out = w3^T h2                         PE matmul (f32r), w3 zero-padded to
                                          [128,32] and col-tiled so 4 k-tiles
                                          pack one full [128,512] PSUM bank
    one DVE copy PSUM->SBUF staging per 4 tiles, large strided DMAs out.

Sharding: 8 cores x (batch, 256 queries). All weights replicated.
"""

import numpy as np

B, NQ, NK, H, HD = 2, 1024, 1024, 8, 128
NCORES = 8
CPB = NCORES // B          # cores per batch = 4
QPC = NQ // CPB            # queries per core = 256
KT = 512                   # k-tile (matmul moving free dim)
KH = NK // KT              # k halves = 2
NGR = 16                   # groups per staging round (group = 2 q = 4 tiles)
RQ = 2 * NGR               # queries per staging round = 32
W3P = 32                   # w3 padded to 32 output columns (col-group width)

_CACHE = {}


def _build_nc():
    from contextlib import ExitStack

    import concourse.bass as bass
    import concourse.tile as tile
    from concourse import bacc, mybir
    from concourse.alu_op_type import AluOpType

    f32 = mybir.dt.float32
    f32r = mybir.dt.float32r
    bf16 = mybir.dt.bfloat16
    Relu = mybir.ActivationFunctionType.Relu

    nc = bacc.Bacc(
        "TRN2",
        target_bir_lowering=False,
        debug=False,
        enable_asserts=True,
        num_devices=NCORES,
    )

    gamma_d = nc.dram_tensor("gamma", (HD, NK), f32, kind="ExternalInput").ap()
    beta_d = nc.dram_tensor("beta", (HD, QPC), f32, kind="ExternalInput").ap()
    w2_d = nc.dram_tensor("w2", (HD, HD), f32, kind="ExternalInput").ap()
    w3p_d = nc.dram_tensor("w3p", (HD, W3P), f32, kind="ExternalInput").ap()
    b2_d = nc.dram_tensor("b2", (HD, 1), f32, kind="ExternalInput").ap()
    out_d = nc.dram_tensor("out", (H, QPC, NK), f32, kind="ExternalOutput").ap()

    with tile.TileContext(nc) as tc:
        with ExitStack() as ctx:
            consts = ctx.enter_context(tc.tile_pool(name="consts", bufs=1))
            h1p = ctx.enter_context(tc.tile_pool(name="h1p", bufs=4))
            h2p = ctx.enter_context(tc.tile_pool(name="h2p", bufs=4))
            stagep = ctx.enter_context(tc.tile_pool(name="stagep", bufs=2))
            # ps2 tiles are [128,1024] = 2 banks; 3 bufs + 2 ps3 banks = 8
            ps2 = ctx.enter_context(tc.tile_pool(name="ps2", bufs=3, space="PSUM"))
            ps3 = ctx.enter_context(tc.tile_pool(name="ps3", bufs=2, space="PSUM"))

            gamma = consts.tile([HD, NK], f32)
            nc.sync.dma_start(gamma, gamma_d)
            beta = consts.tile([HD, QPC], f32)
            nc.sync.dma_start(beta, beta_d)
            w2 = consts.tile([HD, HD], f32)
            nc.sync.dma_start(w2, w2_d)
            w3p = consts.tile([HD, W3P], f32)
            nc.sync.dma_start(w3p, w3p_d)
            b2 = consts.tile([HD, 1], f32)
            nc.sync.dma_start(b2, b2_d)

            # walrus requires f32r matmul operands to be *produced* as f32r
            # (rounded on write), so convert weights once and write h1/h2
            # tiles as f32r directly.
            # All matmul operands in bf16: f32r matmuls lower to the 2-pass
            # fp32_mode=HIGH path (~3x slower) and reject col-tiling; DVE
            # writes with f32r dst take a ~5us slow path.  PSUM accumulation
            # stays fp32, so only operand rounding (~2^-9) is lost.
            w2r = consts.tile([HD, HD], bf16)
            nc.vector.tensor_copy(w2r, w2)
            w3pr = consts.tile([HD, W3P], bf16)
            nc.vector.tensor_copy(w3pr, w3p)

            nrounds = QPC // RQ
            for r in range(nrounds):
                q0 = r * RQ
                stage = stagep.tile([128, NGR * KT], f32, tag="stage")
                for g in range(NGR):
                    q = q0 + 2 * g
                    p3 = ps3.tile([128, KT], f32, tag="p3")
                    for jq in range(2):
                        qq = q + jq
                        # h1 for both k-halves of this query in one DVE op
                        h1 = h1p.tile([HD, NK], bf16, tag="h1")
                        nc.vector.tensor_scalar(
                            h1,
                            gamma,
                            beta[:, qq:qq + 1],
                            0.0,
                            AluOpType.add,
                            AluOpType.max,
                        )
                        p2 = ps2.tile([HD, NK], f32, tag="p2")
                        for kh in range(KH):
                            nc.tensor.matmul(
                                p2[:, kh * KT:(kh + 1) * KT],
                                w2r,
                                h1[:, kh * KT:(kh + 1) * KT],
                                start=True,
                                stop=True,
                            )
                        h2 = h2p.tile([HD, NK], bf16, tag="h2")
                        nc.scalar.activation(h2, p2, Relu, bias=b2)
                        for kh in range(KH):
                            j = 2 * jq + kh
                            nc.tensor.matmul(
                                p3[32 * j:32 * (j + 1), :],
                                w3pr,
                                h2[:, kh * KT:(kh + 1) * KT],
                                start=True,
                                stop=True,
                                tile_position=(0, 32 * j),
                            )
                    nc.vector.tensor_copy(stage[:, g * KT:(g + 1) * KT], p3)
                # Staging layout: free slot g holds the 4-tile group for
                # queries (q0+2g, q0+2g+1); partition block 32j..32j+8 holds
                # heads for (q offset j//2, k half j%2).  DMA per block j:
                # dest[n, g, k] = out[n, q0 + 2g + j//2, (j%2)*512 + k]
                for j in range(4):
                    dest = bass.AP(
                        tensor=out_d.tensor,
                        offset=out_d.offset
                        + (q0 + (j // 2)) * NK
                        + (j % 2) * KT,
                        ap=[[QPC * NK, H], [2 * NK, NGR], [1, KT]],
                    )
                    nc.sync.dma_start(dest, stage[32 * j:32 * j + H, :])

    nc.compile()
    return nc


def _get_nc():
    if "nc" not in _CACHE:
        _CACHE["nc"] = _build_nc()
    return _CACHE["nc"]


def make_in_maps(query_coords, key_coords, w1, b1, w2, b2, w3):
    """Host-side shard prep: per-core gamma/beta + replicated weights."""
    qc = np.asarray(query_coords, np.float32)
    kc = np.asarray(key_coords, np.float32)
    w1 = np.asarray(w1, np.float32)
    b1 = np.asarray(b1, np.float32)
    w2 = np.asarray(w2, np.float32)
    b2 = np.asarray(b2, np.float32)
    w3 = np.asarray(w3, np.float32)

    w3p = np.zeros((HD, W3P), np.float32)
    w3p[:, :H] = w3
    b2c = np.ascontiguousarray(b2.reshape(HD, 1))
    w2c = np.ascontiguousarray(w2)

    in_maps = []
    for c in range(NCORES):
        b = c // CPB
        q0 = (c % CPB) * QPC
        gamma = np.ascontiguousarray(-(kc[b] @ w1).T)            # (128, NK)
        beta = np.ascontiguousarray(
            (qc[b, q0:q0 + QPC] @ w1).T + b1[:, None]            # (128, QPC)
        )
        in_maps.append(
            {"gamma": gamma, "beta": beta, "w2": w2c, "w3p": w3p, "b2": b2c}
        )
    return in_maps


def assemble_output(results, b3):
    """Gather per-core [H, QPC, NK] results into (B, H, NQ, NK)."""
    b3 = np.asarray(b3, np.float32)
    out = np.empty((B, H, NQ, NK), np.float32)
    for c in range(NCORES):
        b = c // CPB
        q0 = (c % CPB) * QPC
        out[b, :, q0:q0 + QPC, :] = results[c]["out"]
    if np.any(b3):
        out += b3.reshape(1, H, 1, 1)
    return out


def kernel(**inputs):
    from concourse.bass_utils import run_bass_kernel_spmd

    in_maps = make_in_maps(
        inputs["query_coords"],
        inputs["key_coords"],
        inputs["w1"],
        inputs["b1"],
        inputs["w2"],
        inputs["b2"],
        inputs["w3"],
    )
    nc = _get_nc()
    res = run_bass_kernel_spmd(nc, in_maps, list(range(NCORES)))
    return assemble_output(res.results, inputs["b3"])
